# revision 28
# baseline (speedup 1.0000x reference)
"""AttentionTSP (B=1024,S=256,D=128,FF=2048) sampling decode on 8 Trainium2 cores.

Strategy (pure data parallel, batch sharded 128/core):
  * jax.random.categorical(key42) == argmax(logits + Gumbel) where the Gumbel
    table is a fixed constant -> precomputed on host CPU (bit-exact threefry).
  * query at step t>=1 depends only on (chosen_0, chosen_{t-1}):
      qp[b, j] = C0[b] + Mr @ h[b, j],  Mr = pq_w @ vwe_w[:, D:]
    so the whole decode collapses to a precomputed table
      u[b, j, s] = sum_d pv[d] * tanh(ref_proj[b,s,d] + qp[b,j,d])
    (u rows; logits row = 10*tanh(u row) applied after gather) plus a light
    sequential loop: gather row j_prev -> mask -> +Gumbel -> argmax.
  * log-softmax bits (max, sumexp, chosen logit) accumulated per step,
    finalized in one batch at the end.

Engines: inner tanh on ScalarE (the roofline, ~1.07e9 elems/core); the
ref+qp broadcast-adds split across VectorE/GpSimd; pv reduction on TensorE
(fp32r, M=1 rows packed 4-per-psum-tile at partitions 0/32/64/96).
"""
import sys, os
for p in ('/opt/trn_rl_repo',):
    if p not in sys.path and os.path.isdir(p):
        sys.path.append(p)

import numpy as np

B, S, D, FF = 1024, 256, 128, 2048
NCORES = 8
BC = B // NCORES            # 128 batches per core
CLIP, NEG = 10.0, -100000.0
ROWS = 257                  # table rows per batch: j=0..255 plus j=256 (step-0 row)
DB8 = 2                     # of each 4 groups-of-8j, this many adds on DVE (rest GpSimd)

_CACHE = {}


def _host_prep(inputs_np):
    """Gumbel table + parameter folds, on host CPU jax (bit-exact vs reference)."""
    import jax, jax.numpy as jnp
    cpu = jax.devices("cpu")[0]
    with jax.default_device(cpu):
        if "G" not in _CACHE:
            keys = jax.random.split(jax.random.key(42), S)
            gfn = jax.jit(lambda k: jax.random.gumbel(k, (B, S), jnp.float32))
            G = np.stack([np.asarray(gfn(keys[t])) for t in range(S)])
            _CACHE["G"] = G  # [S, B, S]
        G = _CACHE["G"]

        f32 = np.float32
        p = {k: np.asarray(v, f32) for k, v in inputs_np.items() if k != "inputs"}
        jf = lambda x: jnp.asarray(x, jnp.float32)

        w = {}
        ipw = p["in_proj_w"]
        w["wq"] = ipw[0:D].T.copy()
        w["wk"] = ipw[D:2 * D].T.copy()
        w["wv"] = ipw[2 * D:3 * D].T.copy()
        w["wo"] = p["out_proj_w"].T.copy()
        w["w1"] = p["ff1_w"].T.copy()                       # [128, 2048] lhsT chunks
        w["w2"] = p["ff2_w"].T.reshape(16, 128, 128).transpose(1, 0, 2).copy()  # [128,16,128]
        w["prw"] = p["pr_w"].T.copy()
        w["hctx"] = (p["hctx_w"] / 256.0).T.copy()          # fold exact 2^-8 mean
        w["pqw"] = p["pq_w"].T.copy()
        M1 = np.asarray(jf(p["pq_w"]) @ jf(p["vwe_w"][:, :D]), f32)
        Mr = np.asarray(jf(p["pq_w"]) @ jf(p["vwe_w"][:, D:]), f32)
        w["m1"] = M1.T.copy()
        w["mr"] = Mr.T.copy()
        w["embw"] = p["embed_w"].copy()                     # [2, 128]

        init_proj = np.asarray(jf(p["init_w"]) @ jf(p["vwe_w"]).T + jf(p["vwe_b"]), f32)
        vec_qp0 = np.asarray(jf(init_proj) @ jf(p["pq_w"]).T + jf(p["pq_b"]), f32)
        vec_c0 = np.asarray(jf(p["vwe_b"]) @ jf(p["pq_w"]).T + jf(p["pq_b"]), f32)

        ipb = p["in_proj_b"]
        vec_cols = [
            ipb[0:D], ipb[D:2 * D], ipb[2 * D:3 * D],       # 0 bq, 1 bk, 2 bv
            p["out_proj_b"],                                 # 3 bo
            p["ln1_g"], p["ln1_b"], p["ln2_g"], p["ln2_b"],  # 4..7
            p["pr_b"], p["pv"],                              # 8, 9
            vec_qp0, vec_c0,                                 # 10, 11
            p["hctx_b"], p["ff2_b"],                         # 12, 13
        ]
        w["vecs"] = np.stack(vec_cols, axis=1).astype(f32)   # [128, 14]
        w["fb1"] = p["ff1_b"].reshape(16, 128).T.copy()      # [128, 16]
        return G, w


def _build(nc_mod):
    PH = os.environ.get("PHASES", "ABCDE")
    """Build the Bass module once. Returns nc."""
    bass, mybir, tile = nc_mod
    FP = mybir.dt.float32
    FR = mybir.dt.float32r
    I32 = mybir.dt.int32
    U32 = mybir.dt.uint32
    AF = mybir.ActivationFunctionType
    AL = mybir.AluOpType
    from concourse.masks import make_identity

    # Cap DMA sem lanes so phase-boundary/drain instructions wait on few
    # queue semaphores (this walrus build has a small per-instruction
    # sync-wait budget).
    from concourse import bacc
    nc = bacc.Bacc("TRN2", target_bir_lowering=False, debug=False)

    xT_t = nc.dram_tensor("xT", [BC, 2, S], FP, kind="ExternalInput")
    G_t = nc.dram_tensor("G", [S, BC, S], FP, kind="ExternalInput")
    wq_t = nc.dram_tensor("wq", [D, D], FP, kind="ExternalInput")
    wk_t = nc.dram_tensor("wk", [D, D], FP, kind="ExternalInput")
    wv_t = nc.dram_tensor("wv", [D, D], FP, kind="ExternalInput")
    wo_t = nc.dram_tensor("wo", [D, D], FP, kind="ExternalInput")
    w1_t = nc.dram_tensor("w1", [D, FF], FP, kind="ExternalInput")
    w2_t = nc.dram_tensor("w2", [D, 16, D], FP, kind="ExternalInput")
    prw_t = nc.dram_tensor("prw", [D, D], FP, kind="ExternalInput")
    hctx_t = nc.dram_tensor("hctx", [D, D], FP, kind="ExternalInput")
    pqw_t = nc.dram_tensor("pqw", [D, D], FP, kind="ExternalInput")
    m1_t = nc.dram_tensor("m1", [D, D], FP, kind="ExternalInput")
    mr_t = nc.dram_tensor("mr", [D, D], FP, kind="ExternalInput")
    embw_t = nc.dram_tensor("embw", [2, D], FP, kind="ExternalInput")
    vecs_t = nc.dram_tensor("vecs", [D, 14], FP, kind="ExternalInput")
    fb1_t = nc.dram_tensor("fb1", [D, 16], FP, kind="ExternalInput")

    TBL = nc.dram_tensor("TBL", [BC * ROWS, S], FP, kind="Internal")
    HTOK = nc.dram_tensor("HTOK", [BC * S, D], FP, kind="Internal")
    HRD = nc.dram_tensor("HRD", [BC, D, 2 * S], FP, kind="Internal")

    lp_t = nc.dram_tensor("lp", [BC, S], FP, kind="ExternalOutput")
    idx_t = nc.dram_tensor("idx", [BC, S], I32, kind="ExternalOutput")

    INV_SQRT_D = float(1.0 / np.sqrt(np.float32(D)))

    with tile.TileContext(nc) as tc:
        with tc.tile_pool(name="persist", bufs=1) as pers:
            # ---- persistent tiles ----
            wq = pers.tile([D, D], FP); nc.sync.dma_start(wq[:], wq_t.ap())
            wk = pers.tile([D, D], FP); nc.sync.dma_start(wk[:], wk_t.ap())
            wv = pers.tile([D, D], FP); nc.sync.dma_start(wv[:], wv_t.ap())
            wo = pers.tile([D, D], FP); nc.sync.dma_start(wo[:], wo_t.ap())
            w1 = pers.tile([D, FF], FP); nc.sync.dma_start(w1[:], w1_t.ap())
            w2 = pers.tile([D, 16 * D], FP)
            nc.sync.dma_start(w2[:], w2_t.ap().rearrange("p a b -> p (a b)"))
            prw = pers.tile([D, D], FP); nc.sync.dma_start(prw[:], prw_t.ap())
            hctx = pers.tile([D, D], FP); nc.sync.dma_start(hctx[:], hctx_t.ap())
            pqw = pers.tile([D, D], FP); nc.sync.dma_start(pqw[:], pqw_t.ap())
            m1w = pers.tile([D, D], FP); nc.sync.dma_start(m1w[:], m1_t.ap())
            mrw = pers.tile([D, D], FP); nc.sync.dma_start(mrw[:], mr_t.ap())
            embw = pers.tile([2, D], FP); nc.sync.dma_start(embw[:], embw_t.ap())
            vecs = pers.tile([D, 14], FP); nc.sync.dma_start(vecs[:], vecs_t.ap())
            fb1 = pers.tile([D, 16], FP); nc.sync.dma_start(fb1[:], fb1_t.ap())

            ident = pers.tile([D, D], FP)
            make_identity(nc, ident[:])
            inv128 = pers.tile([D, 1], FP)
            nc.vector.memset(inv128[:], 1.0 / 128.0)
            ones1 = pers.tile([1, D], FP)
            nc.vector.memset(ones1[:], 1.0)

            pvm = pers.tile([D, 2 * D], FP)
            nc.vector.memset(pvm[:], 0.0)
            nc.vector.tensor_copy(pvm[:, D:D + 1], vecs[:, 9:10])

            iota_i = pers.tile([BC, S], I32)
            nc.gpsimd.iota(iota_i[:], pattern=[[1, S]], base=0, channel_multiplier=0)
            iota_f = pers.tile([BC, S], FP)
            nc.vector.tensor_copy(iota_f[:], iota_i[:])
            iota257 = pers.tile([BC, 1], I32)
            nc.gpsimd.iota(iota257[:], pattern=[[0, 1]], base=0, channel_multiplier=ROWS)
            iota256 = pers.tile([BC, 1], I32)
            nc.gpsimd.iota(iota256[:], pattern=[[0, 1]], base=0, channel_multiplier=S)
            idx0 = pers.tile([BC, 1], I32)
            nc.gpsimd.iota(idx0[:], pattern=[[0, 1]], base=S, channel_multiplier=ROWS)

            # stash across phases
            hbp_all = pers.tile([D, BC], FP)      # pq_w @ h_bar^T columns
            c0_all = pers.tile([D, BC], FP)
            maskadd = pers.tile([BC, S], FP)
            SE = pers.tile([BC, S], FP)
            MXN = pers.tile([BC, S], FP)          # negated row max
            CL = pers.tile([BC, S], FP)           # chosen logit
            idx_acc = pers.tile([BC, S], I32)

            bq, bk, bv, bo = (vecs[:, i:i + 1] for i in range(4))
            g1, b1, g2, b2 = (vecs[:, i:i + 1] for i in range(4, 8))
            prb, pv = vecs[:, 8:9], vecs[:, 9:10]
            vqp0, vc0, hctxb, fb2 = (vecs[:, i:i + 1] for i in range(10, 14))

            def ln_feature_major(pool, psum, x_pre, gcol, bcol, out=None):
                """LayerNorm over partition dim (d) for [128, 256] feature-major x."""
                m_ps = psum.tile([1, S], FP, tag="row", bufs=2)
                nc.tensor.matmul(m_ps[:], inv128[:], x_pre[:], start=True, stop=True)
                m_row = pool.tile([1, S], FP, tag="r1")
                nc.vector.tensor_copy(m_row[:], m_ps[:])
                mo_ps = psum.tile([D, S], FP, tag="big", bufs=2)
                nc.tensor.matmul(mo_ps[:], ones1[:], m_row[:], start=True, stop=True)
                xc = pool.tile([D, S], FP, tag="xc")
                nc.vector.tensor_sub(xc[:], x_pre[:], mo_ps[:])
                sq = pool.tile([D, S], FP, tag="sq")
                nc.scalar.square(sq[:], xc[:])
                v_ps = psum.tile([1, S], FP, tag="row", bufs=2)
                nc.tensor.matmul(v_ps[:], inv128[:], sq[:], start=True, stop=True)
                # rs = 1/sqrt(v+eps), sqrt refined with 2 Newton steps on gpsimd
                veps = pool.tile([1, S], FP, tag="r2")
                nc.vector.tensor_scalar(out=veps[:], in0=v_ps[:], scalar1=1e-5,
                                        scalar2=None, op0=AL.add)
                s0 = pool.tile([1, S], FP, tag="r3")
                nc.scalar.sqrt(s0[:], veps[:])
                rs = pool.tile([1, S], FP, tag="r6")
                nc.vector.reciprocal(rs[:], s0[:])
                t1 = pool.tile([1, S], FP, tag="r4")
                t2 = pool.tile([1, S], FP, tag="r5")
                for _ in range(2):
                    nc.gpsimd.tensor_tensor(out=t1[:], in0=rs[:], in1=rs[:], op=AL.mult)
                    nc.gpsimd.tensor_tensor(out=t2[:], in0=t1[:], in1=veps[:], op=AL.mult)
                    nc.vector.tensor_scalar(out=t2[:], in0=t2[:], scalar1=-0.5,
                                            scalar2=1.5, op0=AL.mult, op1=AL.add)
                    nc.gpsimd.tensor_tensor(out=rs[:], in0=rs[:], in1=t2[:], op=AL.mult)
                rso_ps = psum.tile([D, S], FP, tag="big", bufs=2)
                nc.tensor.matmul(rso_ps[:], ones1[:], rs[:], start=True, stop=True)
                xn = pool.tile([D, S], FP, tag="xn")
                nc.vector.tensor_mul(xn[:], xc[:], rso_ps[:])
                if out is None:
                    xout = pool.tile([D, S], FP, tag="xo")
                    out = xout[:]
                nc.vector.tensor_scalar(out=out, in0=xn[:], scalar1=gcol,
                                        scalar2=bcol, op0=AL.mult, op1=AL.add)
                return out

            # ================= PHASE A: encoder per batch =================
            if "A" in PH:
             with nc.named_scope("phaseA"), \
                 tc.tile_pool(name="pa", bufs=2) as pa, \
                 tc.tile_pool(name="pap", bufs=1, space="PSUM") as pap:
              for iv in range(BC):
                xT = pa.tile([2, S], FP)
                nc.sync.dma_start(xT[:], xT_t.ap()[bass.ds(iv, 1)].squeeze(0))
                emb_ps = pap.tile([D, S], FP, tag="big", bufs=2)
                nc.tensor.matmul(emb_ps[:], embw[:], xT[:], start=True, stop=True)
                embT = pa.tile([D, S], FP, tag="embT")
                nc.scalar.copy(embT[:], emb_ps[:])

                qkv = {}
                for nm, wt, bcol in (("q", wq, bq), ("k", wk, bk), ("v", wv, bv)):
                    ps = pap.tile([D, S], FP, tag="big", bufs=2)
                    nc.tensor.matmul(ps[:], wt[:], embT[:], start=True, stop=True)
                    sb = pa.tile([D, S], FP, tag=f"{nm}T")
                    nc.scalar.activation(sb[:], ps[:], AF.Identity, bias=bcol)
                    qkv[nm] = sb

                # v to token-major via PE transpose
                vtok = []
                for c in range(2):
                    tp = pap.tile([D, D], FP, tag="sq")
                    nc.tensor.transpose(tp[:], qkv["v"][:, c * D:(c + 1) * D], ident[:])
                    vt = pa.tile([D, D], FP, tag=f"vtok{c}")
                    nc.vector.tensor_copy(vt[:], tp[:])
                    vtok.append(vt)

                attnT = []
                attn_sb = []
                for c in range(2):
                    sc_ps = pap.tile([D, S], FP, tag="big", bufs=2)
                    nc.tensor.matmul(sc_ps[:], qkv["q"][:, c * D:(c + 1) * D],
                                     qkv["k"][:], start=True, stop=True)
                    scl = pa.tile([D, S], FP, tag=f"scl{c}")
                    nc.vector.tensor_scalar(out=scl[:], in0=sc_ps[:], scalar1=INV_SQRT_D,
                                            scalar2=None, op0=AL.mult)
                    ngm = pa.tile([D, 1], FP, tag=f"ngm{c}")
                    nc.vector.tensor_reduce(ngm[:], scl[:], axis=mybir.AxisListType.X,
                                            op=AL.max, negate=True)
                    den = pa.tile([D, 1], FP, tag=f"den{c}")
                    ex = pa.tile([D, S], FP, tag=f"ex{c}")
                    nc.scalar.activation(ex[:], scl[:], AF.Exp, bias=ngm[:, :1],
                                         accum_out=den[:])
                    dre = pa.tile([D, 1], FP, tag=f"dre{c}")
                    nc.vector.reciprocal(dre[:], den[:])
                    at = pa.tile([D, S], FP, tag=f"at{c}")
                    nc.vector.tensor_scalar(out=at[:], in0=ex[:], scalar1=dre[:, :1],
                                            scalar2=None, op0=AL.mult)
                    attn_sb.append(at)
                for ct in range(2):
                    tp = pap.tile([D, S], FP, tag="big", bufs=2)
                    for cs in range(2):
                        nc.tensor.transpose(tp[:, cs * D:(cs + 1) * D],
                                            attn_sb[cs][:, ct * D:(ct + 1) * D], ident[:])
                    att = pa.tile([D, S], FP, tag=f"attnT{ct}")
                    nc.vector.tensor_copy(att[:], tp[:])
                    attnT.append(att)

                ao_ps = pap.tile([D, S], FP, tag="big", bufs=2)
                for c in range(2):
                    nc.tensor.matmul(ao_ps[:], vtok[c][:],
                                     attnT[c][:], start=(c == 0), stop=(c == 1))
                aoT = pa.tile([D, S], FP, tag="aoT")
                nc.scalar.copy(aoT[:], ao_ps[:])
                aop_ps = pap.tile([D, S], FP, tag="big", bufs=2)
                nc.tensor.matmul(aop_ps[:], wo[:], aoT[:], start=True, stop=True)
                x_pre = pa.tile([D, S], FP, tag="x_pre")
                nc.vector.scalar_tensor_tensor(out=x_pre[:], in0=aop_ps[:], scalar=bo,
                                               in1=embT[:], op0=AL.add, op1=AL.add)

                x1 = ln_feature_major(pa, pap, x_pre, g1, b1)

                h2_ps = pap.tile([D, S], FP, tag="big2")
                for c in range(16):
                    r_ps = pap.tile([D, S], FP, tag="big", bufs=2)
                    nc.tensor.matmul(r_ps[:], w1[:, c * D:(c + 1) * D],
                                     x1[:], start=True, stop=True)
                    r_sb = pa.tile([D, S], FP, tag="r_sb")
                    nc.vector.tensor_scalar(out=r_sb[:], in0=r_ps[:],
                                            scalar1=fb1[:, c:c + 1], scalar2=0.0,
                                            op0=AL.add, op1=AL.max)
                    nc.tensor.matmul(h2_ps[:], w2[:, c * D:(c + 1) * D],
                                     r_sb[:], start=(c == 0), stop=(c == 15))
                x2_pre = pa.tile([D, S], FP, tag="x2_pre")
                nc.vector.scalar_tensor_tensor(out=x2_pre[:], in0=h2_ps[:], scalar=fb2,
                                               in1=x1[:], op0=AL.add, op1=AL.add)
                hr = pa.tile([D, 2 * S], FP, tag="hr")
                hT = ln_feature_major(pa, pap, x2_pre, g2, b2, out=hr[:, 0:S])

                # token-major h rows for first_hs gather
                htok = pa.tile([D, 2 * D], FP, tag="htok")
                for c in range(2):
                    tp = pap.tile([D, D], FP, tag="sq")
                    nc.tensor.transpose(tp[:], hT[:, c * D:(c + 1) * D], ident[:])
                    nc.vector.tensor_copy(htok[:, c * D:(c + 1) * D], tp[:])
                hta = HTOK.ap()
                nc.sync.dma_start(
                    bass.AP(hta.tensor, (iv * S) * D, [[D, D], [D * D, 2], [1, D]]),
                    htok[:].rearrange("p (c d) -> p c d", c=2))

                # ref_proj^T
                rp_ps = pap.tile([D, S], FP, tag="big", bufs=2)
                nc.tensor.matmul(rp_ps[:], prw[:], hT, start=True, stop=True)
                refT = hr[:, S:2 * S]
                nc.scalar.activation(refT, rp_ps[:], AF.Identity, bias=prb)
                nc.sync.dma_start(HRD.ap()[bass.ds(iv, 1)].squeeze(0), hr[:])

                # h_bar projection column + step-0 u row
                sumc = pa.tile([D, 1], FP, tag="sumc")
                nc.vector.tensor_reduce(sumc[:], hT, axis=mybir.AxisListType.X,
                                        op=AL.add)
                hb_ps = pap.tile([D, 1], FP, tag="col")
                nc.tensor.matmul(hb_ps[:], hctx[:], sumc[:], start=True, stop=True)
                hbar = pa.tile([D, 1], FP, tag="hbar")
                nc.scalar.activation(hbar[:], hb_ps[:], AF.Identity, bias=hctxb)
                hbp_ps = pap.tile([D, 1], FP, tag="col")
                nc.tensor.matmul(hbp_ps[:], pqw[:], hbar[:], start=True, stop=True)
                nc.vector.tensor_copy(hbp_all[:, bass.ds(iv, 1)], hbp_ps[:])
                qp0 = pa.tile([D, 1], FP, tag="qp0")
                nc.vector.tensor_scalar(out=qp0[:], in0=hbp_ps[:], scalar1=vqp0,
                                        scalar2=None, op0=AL.add)
                t0 = pa.tile([D, S], FP, tag="t0")
                nc.scalar.activation(t0[:], refT, AF.Tanh, bias=qp0[:, :1])
                u0_ps = pap.tile([1, S], FP, tag="row", bufs=2)
                nc.tensor.matmul(u0_ps[:], pv, t0[:],
                                 start=True, stop=True)
                u0 = pa.tile([1, S], FP, tag="u0")
                nc.vector.tensor_copy(u0[:], u0_ps[:])
                nc.sync.dma_start(TBL.ap()[bass.ds(iv * ROWS + S, 1)], u0[:])

            # ================= PHASE B: step 0 + C0 =================
            if "B" in PH:
             with nc.named_scope("phaseB"), \
                 tc.tile_pool(name="pb", bufs=1) as pb, \
                 tc.tile_pool(name="pbp", bufs=2, space="PSUM") as pbp:
                row0 = pb.tile([BC, S], FP)
                nc.gpsimd.indirect_dma_start(
                    out=row0[:], out_offset=None, in_=TBL.ap(),
                    in_offset=bass.IndirectOffsetOnAxis(ap=idx0[:, :1], axis=0))
                l0 = pb.tile([BC, S], FP)
                nc.scalar.activation(l0[:], row0[:], AF.Tanh)
                nc.vector.tensor_scalar(out=l0[:], in0=l0[:], scalar1=CLIP,
                                        scalar2=None, op0=AL.mult)
                g0 = pb.tile([BC, S], FP)
                nc.sync.dma_start(g0[:], G_t.ap()[0])
                sc0 = pb.tile([BC, S], FP)
                nc.vector.tensor_add(sc0[:], l0[:], g0[:])
                mv8 = pb.tile([BC, 8], FP)
                mi8 = pb.tile([BC, 8], U32)
                nc.vector.max_with_indices(mv8[:], mi8[:], sc0[:])
                ch_i = pers.tile([BC, 1], I32, name="ch0")
                nc.vector.tensor_copy(ch_i[:], mi8[:, :1])
                nc.vector.tensor_copy(idx_acc[:, 0:1], mi8[:, :1])
                chf = pb.tile([BC, 1], FP)
                nc.vector.tensor_copy(chf[:], mi8[:, :1])
                eqf = pb.tile([BC, S], FP)
                nc.vector.tensor_scalar(out=eqf[:], in0=iota_f[:], scalar1=chf[:, :1],
                                        scalar2=None, op0=AL.is_equal)
                nc.vector.tensor_scalar(out=maskadd[:], in0=eqf[:], scalar1=NEG,
                                        scalar2=None, op0=AL.mult)
                nc.vector.tensor_reduce(MXN[:, 0:1], l0[:], axis=mybir.AxisListType.X,
                                        op=AL.max, negate=True)
                ejunk = pb.tile([BC, S], FP)
                nc.scalar.activation(ejunk[:], l0[:], AF.Exp, bias=MXN[:, 0:1],
                                     accum_out=SE[:, 0:1])
                junk = pb.tile([BC, S], FP)
                nc.vector.tensor_mul(junk[:], l0[:], eqf[:])
                nc.vector.tensor_reduce(CL[:, 0:1], junk[:],
                                        axis=mybir.AxisListType.X, op=AL.add)

                hidx = pb.tile([BC, 1], I32)
                nc.vector.tensor_tensor(out=hidx[:], in0=iota256[:], in1=ch_i[:],
                                        op=AL.add)
                fhs = pb.tile([BC, D], FP)
                nc.gpsimd.indirect_dma_start(
                    out=fhs[:], out_offset=None, in_=HTOK.ap(),
                    in_offset=bass.IndirectOffsetOnAxis(ap=hidx[:, :1], axis=0))
                fhsT_ps = pbp.tile([D, D], FP)
                nc.tensor.transpose(fhsT_ps[:], fhs[:], ident[:])
                fhsT = pb.tile([D, D], FP)
                nc.vector.tensor_copy(fhsT[:], fhsT_ps[:])
                c0_ps = pbp.tile([D, BC], FP)
                nc.tensor.matmul(c0_ps[:], m1w[:], fhsT[:], start=True, stop=True)
                nc.vector.scalar_tensor_tensor(out=c0_all[:], in0=c0_ps[:], scalar=vc0,
                                               in1=hbp_all[:], op0=AL.add, op1=AL.add)

            # ================= PHASE C: u table =================
            if "C" in PH:
             with nc.named_scope("phaseC"), \
                 tc.tile_pool(name="pc", bufs=2) as pc, \
                 tc.tile_pool(name="pcp", bufs=2, space="PSUM") as pcp, \
                 tc.tile_pool(name="pcq", bufs=2, space="PSUM") as pcq:
              for iv in range(BC):
                hr = pc.tile([D, 2 * S], FP, tag="hr")
                nc.sync.dma_start(hr[:], HRD.ap()[bass.ds(iv, 1)].squeeze(0))
                hT = hr[:, 0:S]
                refT = hr[:, S:2 * S]
                qp_ps = pcq.tile([D, S], FP, tag="qp")
                nc.tensor.matmul(qp_ps[:], mrw[:], hT, start=True, stop=True)
                qp = pc.tile([D, S], FP, tag="qp")
                nc.vector.tensor_scalar(out=qp[:], in0=qp_ps[:],
                                        scalar1=c0_all[:, bass.ds(iv, 1)],
                                        scalar2=None, op0=AL.add)
                U_ps = [pcp.tile([D, 512], FP, tag=f"U{h}", name=f"U{h}") for h in range(2)]
                for jg in range(8):
                    A = pc.tile([D, 32 * S], FP, tag="A")
                    for g8 in range(4):
                        j0 = jg * 32 + g8 * 8
                        eng = nc.vector if g8 < DB8 else nc.gpsimd
                        rin = bass.AP(refT.tensor, refT.offset,
                                      [list(refT.ap[0]), [0, 8], [1, S]])
                        qsl = qp[:, j0:j0 + 8]
                        qin = bass.AP(qsl.tensor, qsl.offset,
                                      [list(qsl.ap[0]), [1, 8], [0, S]])
                        eng.tensor_tensor(out=A[:, g8 * 8 * S:(g8 + 1) * 8 * S]
                                          .rearrange("p (e s) -> p e s", e=8),
                                          in0=rin, in1=qin, op=AL.add)
                    nc.scalar.activation(A[:], A[:], AF.Tanh)
                    # 16 j-pairs: accumulate into psum rows 64*(jg%2)+jp of U_ps[jg//4]
                    ups = U_ps[jg // 4]
                    for jp in range(16):
                        k = 64 * (jg % 4) + jp if False else (jg % 4) * 16 + jp
                        nc.tensor.matmul(
                            ups[:], pvm[:, D - k:2 * D - k],
                            A[:, jp * 512:(jp + 1) * 512],
                            start=(k == 0), stop=(k == 63))
                    if jg % 4 == 3:
                        h = jg // 4
                        u_sb = pc.tile([64, 512], FP, tag="u_sb")
                        nc.vector.tensor_copy(u_sb[:], ups[0:64, :])
                        ta = TBL.ap()
                        nc.sync.dma_start(
                            bass.AP(ta.tensor, (iv * ROWS + h * 128) * S,
                                    [[2 * S, 64], [1, 2 * S]]),
                            u_sb[:])

            # ================= PHASE D: decode steps 1..255 =================
            if "D" in PH:
             with nc.named_scope("phaseD"), \
                 tc.tile_pool(name="pd", bufs=3) as pd, \
                 tc.tile_pool(name="pg", bufs=4) as pg:
                ch_prev = ch_i
                for t in range(1, S):
                    gt = pg.tile([BC, S], FP, tag="g")
                    nc.sync.dma_start(gt[:], G_t.ap()[t])
                    gidx = pd.tile([BC, 1], I32, tag="gidx")
                    nc.vector.tensor_tensor(out=gidx[:], in0=iota257[:], in1=ch_prev[:],
                                            op=AL.add)
                    row = pd.tile([BC, S], FP, tag="row")
                    nc.gpsimd.indirect_dma_start(
                        out=row[:], out_offset=None, in_=TBL.ap(),
                        in_offset=bass.IndirectOffsetOnAxis(ap=gidx[:, :1], axis=0))
                    rt = pd.tile([BC, S], FP, tag="rt")
                    nc.scalar.activation(rt[:], row[:], AF.Tanh)
                    masked = pd.tile([BC, S], FP, tag="masked")
                    nc.vector.scalar_tensor_tensor(out=masked[:], in0=rt[:], scalar=CLIP,
                                                   in1=maskadd[:], op0=AL.mult, op1=AL.add)
                    sc = pd.tile([BC, S], FP, tag="sc")
                    nc.vector.tensor_add(sc[:], masked[:], gt[:])
                    mv = pd.tile([BC, 8], FP, tag="mv")
                    mi = pd.tile([BC, 8], U32, tag="mi")
                    nc.vector.max_with_indices(mv[:], mi[:], sc[:])
                    ch = pd.tile([BC, 1], I32, tag="ch")
                    nc.vector.tensor_copy(ch[:], mi[:, :1])
                    nc.vector.tensor_copy(idx_acc[:, t:t + 1], mi[:, :1])
                    chf = pd.tile([BC, 1], FP, tag="chf")
                    nc.gpsimd.tensor_copy(chf[:], mi[:, :1])
                    eqf = pd.tile([BC, S], FP, tag="eqf")
                    nc.vector.tensor_scalar(out=eqf[:], in0=iota_f[:], scalar1=chf[:, :1],
                                            scalar2=None, op0=AL.is_equal)
                    nc.vector.tensor_reduce(MXN[:, t:t + 1], masked[:],
                                            axis=mybir.AxisListType.X, op=AL.max,
                                            negate=True)
                    ej = pd.tile([BC, S], FP, tag="ej")
                    nc.scalar.activation(ej[:], masked[:], AF.Exp, bias=MXN[:, t:t + 1],
                                         accum_out=SE[:, t:t + 1])
                    jk = pd.tile([BC, S], FP, tag="jk")
                    nc.vector.tensor_mul(jk[:], masked[:], eqf[:])
                    nc.vector.tensor_reduce(CL[:, t:t + 1], jk[:],
                                            axis=mybir.AxisListType.X, op=AL.add)
                    nc.vector.scalar_tensor_tensor(out=maskadd[:], in0=eqf[:], scalar=NEG,
                                                    in1=maskadd[:], op0=AL.mult, op1=AL.add)
                    ch_prev = ch

            # ================= PHASE E: finalize logprobs =================
            if "E" in PH:
             with nc.named_scope("phaseE"), tc.tile_pool(name="pe", bufs=1) as pe:
                lnse = pe.tile([BC, S], FP)
                nc.scalar.activation(lnse[:], SE[:], AF.Ln)
                lp1 = pe.tile([BC, S], FP)
                nc.vector.tensor_add(lp1[:], CL[:], MXN[:])
                lp2 = pe.tile([BC, S], FP)
                nc.vector.tensor_sub(lp2[:], lp1[:], lnse[:])
                nc.sync.dma_start(lp_t.ap(), lp2[:])
                nc.sync.dma_start(idx_t.ap(), idx_acc[:])

    nc.compile()
    return nc


def _get_nc():
    if "nc" not in _CACHE:
        import concourse.bass as bass
        import concourse.mybir as mybir
        import concourse.tile as tile
        _CACHE["nc"] = _build((bass, mybir, tile))
    return _CACHE["nc"]


def kernel(**inputs):
    from concourse import bass_utils

    G, w = _host_prep(inputs)
    nc = _get_nc()

    x = np.asarray(inputs["inputs"], np.float32)          # [B, S, 2]
    in_maps = []
    for c in range(NCORES):
        sl = slice(c * BC, (c + 1) * BC)
        m = {k: w[k] for k in ("wq", "wk", "wv", "wo", "w1", "prw", "hctx", "pqw",
                               "m1", "mr", "embw", "vecs", "fb1")}
        m["w2"] = w["w2"]
        m["xT"] = np.ascontiguousarray(x[sl].transpose(0, 2, 1))
        m["G"] = np.ascontiguousarray(G[:, sl, :])
        in_maps.append(m)

    res = bass_utils.run_bass_kernel_spmd(nc, in_maps, core_ids=list(range(NCORES)))
    lp = np.concatenate([r["lp"] for r in res.results], axis=0)
    idx = np.concatenate([r["idx"] for r in res.results], axis=0)
    return lp, idx.astype(np.int32)


if __name__ == "__main__":
    import reference as ref
    import jax
    with jax.default_device(jax.devices("cpu")[0]):
        inputs = {k: np.asarray(v) for k, v in ref.setup_inputs().items()}
        exp_lp, exp_idx = ref.reference(**inputs)
        exp_lp, exp_idx = np.asarray(exp_lp), np.asarray(exp_idx)
    lp, idx = kernel(**inputs)
    n_bad = int((idx != exp_idx).any(axis=1).sum())
    print(f"rows diverging: {n_bad}/1024")
    rel_lp = np.linalg.norm(lp - exp_lp) / np.linalg.norm(exp_lp)
    rel_ix = np.linalg.norm((idx - exp_idx).astype(np.float64)) / np.linalg.norm(exp_idx.astype(np.float64))
    print(f"rel err: logprobs {rel_lp:.5f} indices {rel_ix:.5f}")
    ok = idx == exp_idx
    print(f"logprob err on matching entries: {np.abs(lp - exp_lp)[ok].max():.3e}")


# revision 30
# speedup vs baseline: 1.0041x; 1.0041x over previous
"""AttentionTSP (B=1024,S=256,D=128,FF=2048) sampling decode on 8 Trainium2 cores.

Strategy (pure data parallel, batch sharded 128/core):
  * jax.random.categorical(key42) == argmax(logits + Gumbel) where the Gumbel
    table is a fixed constant -> precomputed on host CPU (bit-exact threefry).
  * query at step t>=1 depends only on (chosen_0, chosen_{t-1}):
      qp[b, j] = C0[b] + Mr @ h[b, j],  Mr = pq_w @ vwe_w[:, D:]
    so the whole decode collapses to a precomputed table
      u[b, j, s] = sum_d pv[d] * tanh(ref_proj[b,s,d] + qp[b,j,d])
    (u rows; logits row = 10*tanh(u row) applied after gather) plus a light
    sequential loop: gather row j_prev -> mask -> +Gumbel -> argmax.
  * log-softmax bits (max, sumexp, chosen logit) accumulated per step,
    finalized in one batch at the end.

Engines: inner tanh on ScalarE (the roofline, ~1.07e9 elems/core); the
ref+qp broadcast-adds are step-0-stride TensorTensor ops split across
VectorE/GpSimd; the pv reduction runs on TensorE in fp32 via 64 accumulating
matmuls per batch whose lhsT is a shifted slice of one [128,256] tensor
holding pv at column 128 (row k of the psum tile = u row of j-pair k).
Note: fp32r is a ~12-bit-mantissa format on this stack - unusable here.
Toolchain notes: must build with bacc.Bacc + nc.compile() (legalizes multi-
sem waits); For_i back-edges emit IncSwdgeSem/Drain forms this walrus
rejects, hence full unroll; gumbel table must be built per-key (vmap over
keys gives different threefry bits).
"""
import sys, os
for p in ('/opt/trn_rl_repo',):
    if p not in sys.path and os.path.isdir(p):
        sys.path.append(p)

import numpy as np

B, S, D, FF = 1024, 256, 128, 2048
NCORES = 8
BC = B // NCORES            # 128 batches per core
CLIP, NEG = 10.0, -100000.0
ROWS = 257                  # table rows per batch: j=0..255 plus j=256 (step-0 row)
DB8 = 2                     # of each 4 groups-of-8j, this many adds on DVE (rest GpSimd)

_CACHE = {}


def _host_prep(inputs_np):
    """Gumbel table + parameter folds, on host CPU jax (bit-exact vs reference)."""
    import jax, jax.numpy as jnp
    cpu = jax.devices("cpu")[0]
    with jax.default_device(cpu):
        if "G" not in _CACHE:
            keys = jax.random.split(jax.random.key(42), S)
            gfn = jax.jit(lambda k: jax.random.gumbel(k, (B, S), jnp.float32))
            G = np.stack([np.asarray(gfn(keys[t])) for t in range(S)])
            _CACHE["G"] = G  # [S, B, S]
        G = _CACHE["G"]

        f32 = np.float32
        p = {k: np.asarray(v, f32) for k, v in inputs_np.items() if k != "inputs"}
        jf = lambda x: jnp.asarray(x, jnp.float32)

        w = {}
        ipw = p["in_proj_w"]
        w["wq"] = ipw[0:D].T.copy()
        w["wk"] = ipw[D:2 * D].T.copy()
        w["wv"] = ipw[2 * D:3 * D].T.copy()
        w["wo"] = p["out_proj_w"].T.copy()
        w["w1"] = p["ff1_w"].T.copy()                       # [128, 2048] lhsT chunks
        w["w2"] = p["ff2_w"].T.reshape(16, 128, 128).transpose(1, 0, 2).copy()  # [128,16,128]
        w["prw"] = p["pr_w"].T.copy()
        w["hctx"] = (p["hctx_w"] / 256.0).T.copy()          # fold exact 2^-8 mean
        w["pqw"] = p["pq_w"].T.copy()
        M1 = np.asarray(jf(p["pq_w"]) @ jf(p["vwe_w"][:, :D]), f32)
        Mr = np.asarray(jf(p["pq_w"]) @ jf(p["vwe_w"][:, D:]), f32)
        w["m1"] = M1.T.copy()
        w["mr"] = Mr.T.copy()
        w["embw"] = p["embed_w"].copy()                     # [2, 128]

        init_proj = np.asarray(jf(p["init_w"]) @ jf(p["vwe_w"]).T + jf(p["vwe_b"]), f32)
        vec_qp0 = np.asarray(jf(init_proj) @ jf(p["pq_w"]).T + jf(p["pq_b"]), f32)
        vec_c0 = np.asarray(jf(p["vwe_b"]) @ jf(p["pq_w"]).T + jf(p["pq_b"]), f32)

        ipb = p["in_proj_b"]
        vec_cols = [
            ipb[0:D], ipb[D:2 * D], ipb[2 * D:3 * D],       # 0 bq, 1 bk, 2 bv
            p["out_proj_b"],                                 # 3 bo
            p["ln1_g"], p["ln1_b"], p["ln2_g"], p["ln2_b"],  # 4..7
            p["pr_b"], p["pv"],                              # 8, 9
            vec_qp0, vec_c0,                                 # 10, 11
            p["hctx_b"], p["ff2_b"],                         # 12, 13
        ]
        w["vecs"] = np.stack(vec_cols, axis=1).astype(f32)   # [128, 14]
        w["fb1"] = p["ff1_b"].reshape(16, 128).T.copy()      # [128, 16]
        return G, w


def _build(nc_mod):
    PH = os.environ.get("PHASES", "ABCDE")
    """Build the Bass module once. Returns nc."""
    bass, mybir, tile = nc_mod
    FP = mybir.dt.float32
    FR = mybir.dt.float32r
    I32 = mybir.dt.int32
    U32 = mybir.dt.uint32
    AF = mybir.ActivationFunctionType
    AL = mybir.AluOpType
    from concourse.masks import make_identity

    # Cap DMA sem lanes so phase-boundary/drain instructions wait on few
    # queue semaphores (this walrus build has a small per-instruction
    # sync-wait budget).
    from concourse import bacc
    nc = bacc.Bacc("TRN2", target_bir_lowering=False, debug=False)

    xT_t = nc.dram_tensor("xT", [BC, 2, S], FP, kind="ExternalInput")
    G_t = nc.dram_tensor("G", [S, BC, S], FP, kind="ExternalInput")
    wq_t = nc.dram_tensor("wq", [D, D], FP, kind="ExternalInput")
    wk_t = nc.dram_tensor("wk", [D, D], FP, kind="ExternalInput")
    wv_t = nc.dram_tensor("wv", [D, D], FP, kind="ExternalInput")
    wo_t = nc.dram_tensor("wo", [D, D], FP, kind="ExternalInput")
    w1_t = nc.dram_tensor("w1", [D, FF], FP, kind="ExternalInput")
    w2_t = nc.dram_tensor("w2", [D, 16, D], FP, kind="ExternalInput")
    prw_t = nc.dram_tensor("prw", [D, D], FP, kind="ExternalInput")
    hctx_t = nc.dram_tensor("hctx", [D, D], FP, kind="ExternalInput")
    pqw_t = nc.dram_tensor("pqw", [D, D], FP, kind="ExternalInput")
    m1_t = nc.dram_tensor("m1", [D, D], FP, kind="ExternalInput")
    mr_t = nc.dram_tensor("mr", [D, D], FP, kind="ExternalInput")
    embw_t = nc.dram_tensor("embw", [2, D], FP, kind="ExternalInput")
    vecs_t = nc.dram_tensor("vecs", [D, 14], FP, kind="ExternalInput")
    fb1_t = nc.dram_tensor("fb1", [D, 16], FP, kind="ExternalInput")

    TBL = nc.dram_tensor("TBL", [BC * ROWS, S], FP, kind="Internal")
    HTOK = nc.dram_tensor("HTOK", [BC * S, D], FP, kind="Internal")
    HRD = nc.dram_tensor("HRD", [BC, D, 2 * S], FP, kind="Internal")

    lp_t = nc.dram_tensor("lp", [BC, S], FP, kind="ExternalOutput")
    idx_t = nc.dram_tensor("idx", [BC, S], I32, kind="ExternalOutput")

    INV_SQRT_D = float(1.0 / np.sqrt(np.float32(D)))

    with tile.TileContext(nc) as tc:
        with tc.tile_pool(name="persist", bufs=1) as pers:
            # ---- persistent tiles ----
            wq = pers.tile([D, D], FP); nc.sync.dma_start(wq[:], wq_t.ap())
            wk = pers.tile([D, D], FP); nc.sync.dma_start(wk[:], wk_t.ap())
            wv = pers.tile([D, D], FP); nc.sync.dma_start(wv[:], wv_t.ap())
            wo = pers.tile([D, D], FP); nc.sync.dma_start(wo[:], wo_t.ap())
            w1 = pers.tile([D, FF], FP); nc.sync.dma_start(w1[:], w1_t.ap())
            w2 = pers.tile([D, 16 * D], FP)
            nc.sync.dma_start(w2[:], w2_t.ap().rearrange("p a b -> p (a b)"))
            prw = pers.tile([D, D], FP); nc.sync.dma_start(prw[:], prw_t.ap())
            hctx = pers.tile([D, D], FP); nc.sync.dma_start(hctx[:], hctx_t.ap())
            pqw = pers.tile([D, D], FP); nc.sync.dma_start(pqw[:], pqw_t.ap())
            m1w = pers.tile([D, D], FP); nc.sync.dma_start(m1w[:], m1_t.ap())
            mrw = pers.tile([D, D], FP); nc.sync.dma_start(mrw[:], mr_t.ap())
            embw = pers.tile([2, D], FP); nc.sync.dma_start(embw[:], embw_t.ap())
            vecs = pers.tile([D, 14], FP); nc.sync.dma_start(vecs[:], vecs_t.ap())
            fb1 = pers.tile([D, 16], FP); nc.sync.dma_start(fb1[:], fb1_t.ap())

            ident = pers.tile([D, D], FP)
            make_identity(nc, ident[:])
            inv128 = pers.tile([D, 1], FP)
            nc.vector.memset(inv128[:], 1.0 / 128.0)
            ones1 = pers.tile([1, D], FP)
            nc.vector.memset(ones1[:], 1.0)

            pvm = pers.tile([D, 2 * D], FP)
            nc.vector.memset(pvm[:], 0.0)
            nc.vector.tensor_copy(pvm[:, D:D + 1], vecs[:, 9:10])

            iota_i = pers.tile([BC, S], I32)
            nc.gpsimd.iota(iota_i[:], pattern=[[1, S]], base=0, channel_multiplier=0)
            iota_f = pers.tile([BC, S], FP)
            nc.vector.tensor_copy(iota_f[:], iota_i[:])
            iota257 = pers.tile([BC, 1], I32)
            nc.gpsimd.iota(iota257[:], pattern=[[0, 1]], base=0, channel_multiplier=ROWS)
            iota256 = pers.tile([BC, 1], I32)
            nc.gpsimd.iota(iota256[:], pattern=[[0, 1]], base=0, channel_multiplier=S)
            idx0 = pers.tile([BC, 1], I32)
            nc.gpsimd.iota(idx0[:], pattern=[[0, 1]], base=S, channel_multiplier=ROWS)

            # stash across phases
            hbp_all = pers.tile([D, BC], FP)      # pq_w @ h_bar^T columns
            c0_all = pers.tile([D, BC], FP)
            maskadd = pers.tile([BC, S], FP)
            SE = pers.tile([BC, S], FP)
            MXN = pers.tile([BC, S], FP)          # negated row max
            CL = pers.tile([BC, S], FP)           # chosen logit
            idx_acc = pers.tile([BC, S], I32)

            bq, bk, bv, bo = (vecs[:, i:i + 1] for i in range(4))
            g1, b1, g2, b2 = (vecs[:, i:i + 1] for i in range(4, 8))
            prb, pv = vecs[:, 8:9], vecs[:, 9:10]
            vqp0, vc0, hctxb, fb2 = (vecs[:, i:i + 1] for i in range(10, 14))

            def ln_feature_major(pool, psum, x_pre, gcol, bcol, out=None):
                """LayerNorm over partition dim (d) for [128, 256] feature-major x."""
                m_ps = psum.tile([1, S], FP, tag="row", bufs=2)
                nc.tensor.matmul(m_ps[:], inv128[:], x_pre[:], start=True, stop=True)
                m_row = pool.tile([1, S], FP, tag="r1")
                nc.vector.tensor_copy(m_row[:], m_ps[:])
                mo_ps = psum.tile([D, S], FP, tag="big", bufs=2)
                nc.tensor.matmul(mo_ps[:], ones1[:], m_row[:], start=True, stop=True)
                xc = pool.tile([D, S], FP, tag="xc")
                nc.vector.tensor_sub(xc[:], x_pre[:], mo_ps[:])
                sq = pool.tile([D, S], FP, tag="sq")
                nc.scalar.square(sq[:], xc[:])
                v_ps = psum.tile([1, S], FP, tag="row", bufs=2)
                nc.tensor.matmul(v_ps[:], inv128[:], sq[:], start=True, stop=True)
                # rs = 1/sqrt(v+eps), sqrt refined with 2 Newton steps on gpsimd
                veps = pool.tile([1, S], FP, tag="r2")
                nc.vector.tensor_scalar(out=veps[:], in0=v_ps[:], scalar1=1e-5,
                                        scalar2=None, op0=AL.add)
                s0 = pool.tile([1, S], FP, tag="r3")
                nc.scalar.sqrt(s0[:], veps[:])
                rs = pool.tile([1, S], FP, tag="r6")
                nc.vector.reciprocal(rs[:], s0[:])
                t1 = pool.tile([1, S], FP, tag="r4")
                t2 = pool.tile([1, S], FP, tag="r5")
                for _ in range(2):
                    nc.gpsimd.tensor_tensor(out=t1[:], in0=rs[:], in1=rs[:], op=AL.mult)
                    nc.gpsimd.tensor_tensor(out=t2[:], in0=t1[:], in1=veps[:], op=AL.mult)
                    nc.vector.tensor_scalar(out=t2[:], in0=t2[:], scalar1=-0.5,
                                            scalar2=1.5, op0=AL.mult, op1=AL.add)
                    nc.gpsimd.tensor_tensor(out=rs[:], in0=rs[:], in1=t2[:], op=AL.mult)
                rso_ps = psum.tile([D, S], FP, tag="big", bufs=2)
                nc.tensor.matmul(rso_ps[:], ones1[:], rs[:], start=True, stop=True)
                xn = pool.tile([D, S], FP, tag="xn")
                nc.vector.tensor_mul(xn[:], xc[:], rso_ps[:])
                if out is None:
                    xout = pool.tile([D, S], FP, tag="xo")
                    out = xout[:]
                nc.vector.tensor_scalar(out=out, in0=xn[:], scalar1=gcol,
                                        scalar2=bcol, op0=AL.mult, op1=AL.add)
                return out

            # ================= PHASE A: encoder per batch =================
            if "A" in PH:
             with nc.named_scope("phaseA"), \
                 tc.tile_pool(name="pa", bufs=2) as pa, \
                 tc.tile_pool(name="pap", bufs=1, space="PSUM") as pap:
              for iv in range(BC):
                xT = pa.tile([2, S], FP)
                nc.sync.dma_start(xT[:], xT_t.ap()[bass.ds(iv, 1)].squeeze(0))
                emb_ps = pap.tile([D, S], FP, tag="big", bufs=2)
                nc.tensor.matmul(emb_ps[:], embw[:], xT[:], start=True, stop=True)
                embT = pa.tile([D, S], FP, tag="embT")
                nc.scalar.copy(embT[:], emb_ps[:])

                qkv = {}
                for nm, wt, bcol in (("q", wq, bq), ("k", wk, bk), ("v", wv, bv)):
                    ps = pap.tile([D, S], FP, tag="big", bufs=2)
                    nc.tensor.matmul(ps[:], wt[:], embT[:], start=True, stop=True)
                    sb = pa.tile([D, S], FP, tag=f"{nm}T")
                    nc.scalar.activation(sb[:], ps[:], AF.Identity, bias=bcol)
                    qkv[nm] = sb

                # v to token-major via PE transpose
                vtok = []
                for c in range(2):
                    tp = pap.tile([D, D], FP, tag="sq")
                    nc.tensor.transpose(tp[:], qkv["v"][:, c * D:(c + 1) * D], ident[:])
                    vt = pa.tile([D, D], FP, tag=f"vtok{c}")
                    nc.vector.tensor_copy(vt[:], tp[:])
                    vtok.append(vt)

                attnT = []
                attn_sb = []
                for c in range(2):
                    sc_ps = pap.tile([D, S], FP, tag="big", bufs=2)
                    nc.tensor.matmul(sc_ps[:], qkv["q"][:, c * D:(c + 1) * D],
                                     qkv["k"][:], start=True, stop=True)
                    scl = pa.tile([D, S], FP, tag=f"scl{c}")
                    nc.vector.tensor_scalar(out=scl[:], in0=sc_ps[:], scalar1=INV_SQRT_D,
                                            scalar2=None, op0=AL.mult)
                    ngm = pa.tile([D, 1], FP, tag=f"ngm{c}")
                    nc.vector.tensor_reduce(ngm[:], scl[:], axis=mybir.AxisListType.X,
                                            op=AL.max, negate=True)
                    den = pa.tile([D, 1], FP, tag=f"den{c}")
                    ex = pa.tile([D, S], FP, tag=f"ex{c}")
                    nc.scalar.activation(ex[:], scl[:], AF.Exp, bias=ngm[:, :1],
                                         accum_out=den[:])
                    dre = pa.tile([D, 1], FP, tag=f"dre{c}")
                    nc.vector.reciprocal(dre[:], den[:])
                    at = pa.tile([D, S], FP, tag=f"at{c}")
                    nc.vector.tensor_scalar(out=at[:], in0=ex[:], scalar1=dre[:, :1],
                                            scalar2=None, op0=AL.mult)
                    attn_sb.append(at)
                for ct in range(2):
                    tp = pap.tile([D, S], FP, tag="big", bufs=2)
                    for cs in range(2):
                        nc.tensor.transpose(tp[:, cs * D:(cs + 1) * D],
                                            attn_sb[cs][:, ct * D:(ct + 1) * D], ident[:])
                    att = pa.tile([D, S], FP, tag=f"attnT{ct}")
                    nc.vector.tensor_copy(att[:], tp[:])
                    attnT.append(att)

                ao_ps = pap.tile([D, S], FP, tag="big", bufs=2)
                for c in range(2):
                    nc.tensor.matmul(ao_ps[:], vtok[c][:],
                                     attnT[c][:], start=(c == 0), stop=(c == 1))
                aoT = pa.tile([D, S], FP, tag="aoT")
                nc.scalar.copy(aoT[:], ao_ps[:])
                aop_ps = pap.tile([D, S], FP, tag="big", bufs=2)
                nc.tensor.matmul(aop_ps[:], wo[:], aoT[:], start=True, stop=True)
                x_pre = pa.tile([D, S], FP, tag="x_pre")
                nc.vector.scalar_tensor_tensor(out=x_pre[:], in0=aop_ps[:], scalar=bo,
                                               in1=embT[:], op0=AL.add, op1=AL.add)

                x1 = ln_feature_major(pa, pap, x_pre, g1, b1)

                h2_ps = pap.tile([D, S], FP, tag="big2")
                for c in range(16):
                    r_ps = pap.tile([D, S], FP, tag="big", bufs=2)
                    nc.tensor.matmul(r_ps[:], w1[:, c * D:(c + 1) * D],
                                     x1[:], start=True, stop=True)
                    r_sb = pa.tile([D, S], FP, tag="r_sb")
                    nc.vector.tensor_scalar(out=r_sb[:], in0=r_ps[:],
                                            scalar1=fb1[:, c:c + 1], scalar2=0.0,
                                            op0=AL.add, op1=AL.max)
                    nc.tensor.matmul(h2_ps[:], w2[:, c * D:(c + 1) * D],
                                     r_sb[:], start=(c == 0), stop=(c == 15))
                x2_pre = pa.tile([D, S], FP, tag="x2_pre")
                nc.vector.scalar_tensor_tensor(out=x2_pre[:], in0=h2_ps[:], scalar=fb2,
                                               in1=x1[:], op0=AL.add, op1=AL.add)
                hr = pa.tile([D, 2 * S], FP, tag="hr")
                hT = ln_feature_major(pa, pap, x2_pre, g2, b2, out=hr[:, 0:S])

                # token-major h rows for first_hs gather
                htok = pa.tile([D, 2 * D], FP, tag="htok")
                for c in range(2):
                    tp = pap.tile([D, D], FP, tag="sq")
                    nc.tensor.transpose(tp[:], hT[:, c * D:(c + 1) * D], ident[:])
                    nc.vector.tensor_copy(htok[:, c * D:(c + 1) * D], tp[:])
                hta = HTOK.ap()
                nc.sync.dma_start(
                    bass.AP(hta.tensor, (iv * S) * D, [[D, D], [D * D, 2], [1, D]]),
                    htok[:].rearrange("p (c d) -> p c d", c=2))

                # ref_proj^T
                rp_ps = pap.tile([D, S], FP, tag="big", bufs=2)
                nc.tensor.matmul(rp_ps[:], prw[:], hT, start=True, stop=True)
                refT = hr[:, S:2 * S]
                nc.scalar.activation(refT, rp_ps[:], AF.Identity, bias=prb)
                nc.sync.dma_start(HRD.ap()[bass.ds(iv, 1)].squeeze(0), hr[:])

                # h_bar projection column + step-0 u row
                sumc = pa.tile([D, 1], FP, tag="sumc")
                nc.vector.tensor_reduce(sumc[:], hT, axis=mybir.AxisListType.X,
                                        op=AL.add)
                hb_ps = pap.tile([D, 1], FP, tag="col")
                nc.tensor.matmul(hb_ps[:], hctx[:], sumc[:], start=True, stop=True)
                hbar = pa.tile([D, 1], FP, tag="hbar")
                nc.scalar.activation(hbar[:], hb_ps[:], AF.Identity, bias=hctxb)
                hbp_ps = pap.tile([D, 1], FP, tag="col")
                nc.tensor.matmul(hbp_ps[:], pqw[:], hbar[:], start=True, stop=True)
                nc.vector.tensor_copy(hbp_all[:, bass.ds(iv, 1)], hbp_ps[:])
                qp0 = pa.tile([D, 1], FP, tag="qp0")
                nc.vector.tensor_scalar(out=qp0[:], in0=hbp_ps[:], scalar1=vqp0,
                                        scalar2=None, op0=AL.add)
                t0 = pa.tile([D, S], FP, tag="t0")
                nc.scalar.activation(t0[:], refT, AF.Tanh, bias=qp0[:, :1])
                u0_ps = pap.tile([1, S], FP, tag="row", bufs=2)
                nc.tensor.matmul(u0_ps[:], pv, t0[:],
                                 start=True, stop=True)
                u0 = pa.tile([1, S], FP, tag="u0")
                nc.vector.tensor_copy(u0[:], u0_ps[:])
                nc.sync.dma_start(TBL.ap()[bass.ds(iv * ROWS + S, 1)], u0[:])

            # ================= PHASE B: step 0 + C0 =================
            if "B" in PH:
             with nc.named_scope("phaseB"), \
                 tc.tile_pool(name="pb", bufs=1) as pb, \
                 tc.tile_pool(name="pbp", bufs=2, space="PSUM") as pbp:
                row0 = pb.tile([BC, S], FP)
                nc.gpsimd.indirect_dma_start(
                    out=row0[:], out_offset=None, in_=TBL.ap(),
                    in_offset=bass.IndirectOffsetOnAxis(ap=idx0[:, :1], axis=0))
                l0 = pb.tile([BC, S], FP)
                nc.scalar.activation(l0[:], row0[:], AF.Tanh)
                nc.vector.tensor_scalar(out=l0[:], in0=l0[:], scalar1=CLIP,
                                        scalar2=None, op0=AL.mult)
                g0 = pb.tile([BC, S], FP)
                nc.sync.dma_start(g0[:], G_t.ap()[0])
                sc0 = pb.tile([BC, S], FP)
                nc.vector.tensor_add(sc0[:], l0[:], g0[:])
                mv8 = pb.tile([BC, 8], FP)
                mi8 = pb.tile([BC, 8], U32)
                nc.vector.max_with_indices(mv8[:], mi8[:], sc0[:])
                ch_i = pers.tile([BC, 1], I32, name="ch0")
                nc.vector.tensor_copy(ch_i[:], mi8[:, :1])
                nc.vector.tensor_copy(idx_acc[:, 0:1], mi8[:, :1])
                chf = pb.tile([BC, 1], FP)
                nc.vector.tensor_copy(chf[:], mi8[:, :1])
                eqf = pb.tile([BC, S], FP)
                nc.vector.tensor_scalar(out=eqf[:], in0=iota_f[:], scalar1=chf[:, :1],
                                        scalar2=None, op0=AL.is_equal)
                nc.vector.tensor_scalar(out=maskadd[:], in0=eqf[:], scalar1=NEG,
                                        scalar2=None, op0=AL.mult)
                nc.vector.tensor_reduce(MXN[:, 0:1], l0[:], axis=mybir.AxisListType.X,
                                        op=AL.max, negate=True)
                ejunk = pb.tile([BC, S], FP)
                nc.scalar.activation(ejunk[:], l0[:], AF.Exp, bias=MXN[:, 0:1],
                                     accum_out=SE[:, 0:1])
                junk = pb.tile([BC, S], FP)
                nc.vector.tensor_mul(junk[:], l0[:], eqf[:])
                nc.vector.tensor_reduce(CL[:, 0:1], junk[:],
                                        axis=mybir.AxisListType.X, op=AL.add)

                hidx = pb.tile([BC, 1], I32)
                nc.vector.tensor_tensor(out=hidx[:], in0=iota256[:], in1=ch_i[:],
                                        op=AL.add)
                fhs = pb.tile([BC, D], FP)
                nc.gpsimd.indirect_dma_start(
                    out=fhs[:], out_offset=None, in_=HTOK.ap(),
                    in_offset=bass.IndirectOffsetOnAxis(ap=hidx[:, :1], axis=0))
                fhsT_ps = pbp.tile([D, D], FP)
                nc.tensor.transpose(fhsT_ps[:], fhs[:], ident[:])
                fhsT = pb.tile([D, D], FP)
                nc.vector.tensor_copy(fhsT[:], fhsT_ps[:])
                c0_ps = pbp.tile([D, BC], FP)
                nc.tensor.matmul(c0_ps[:], m1w[:], fhsT[:], start=True, stop=True)
                nc.vector.scalar_tensor_tensor(out=c0_all[:], in0=c0_ps[:], scalar=vc0,
                                               in1=hbp_all[:], op0=AL.add, op1=AL.add)

            # ================= PHASE C: u table =================
            if "C" in PH:
             with nc.named_scope("phaseC"), \
                 tc.tile_pool(name="pc", bufs=2) as pc, \
                 tc.tile_pool(name="pcp", bufs=2, space="PSUM") as pcp, \
                 tc.tile_pool(name="pcq", bufs=2, space="PSUM") as pcq:
              for iv in range(BC):
                hr = pc.tile([D, 2 * S], FP, tag="hr")
                nc.sync.dma_start(hr[:], HRD.ap()[bass.ds(iv, 1)].squeeze(0))
                hT = hr[:, 0:S]
                refT = hr[:, S:2 * S]
                qp_ps = pcq.tile([D, S], FP, tag="qp")
                nc.tensor.matmul(qp_ps[:], mrw[:], hT, start=True, stop=True)
                qp = pc.tile([D, S], FP, tag="qp")
                nc.vector.tensor_scalar(out=qp[:], in0=qp_ps[:],
                                        scalar1=c0_all[:, bass.ds(iv, 1)],
                                        scalar2=None, op0=AL.add)
                U_ps = [pcp.tile([D, 512], FP, tag=f"U{h}", name=f"U{h}") for h in range(2)]
                for jg in range(8):
                    A = pc.tile([D, 32 * S], FP, tag="A")
                    for g8 in range(4):
                        j0 = jg * 32 + g8 * 8
                        eng = nc.vector if g8 < DB8 else nc.gpsimd
                        rin = bass.AP(refT.tensor, refT.offset,
                                      [list(refT.ap[0]), [0, 8], [1, S]])
                        qsl = qp[:, j0:j0 + 8]
                        qin = bass.AP(qsl.tensor, qsl.offset,
                                      [list(qsl.ap[0]), [1, 8], [0, S]])
                        eng.tensor_tensor(out=A[:, g8 * 8 * S:(g8 + 1) * 8 * S]
                                          .rearrange("p (e s) -> p e s", e=8),
                                          in0=rin, in1=qin, op=AL.add)
                    nc.scalar.activation(A[:], A[:], AF.Tanh)
                    # 16 j-pairs: accumulate into psum rows 64*(jg%2)+jp of U_ps[jg//4]
                    ups = U_ps[jg // 4]
                    for jp in range(16):
                        k = 64 * (jg % 4) + jp if False else (jg % 4) * 16 + jp
                        nc.tensor.matmul(
                            ups[:], pvm[:, D - k:2 * D - k],
                            A[:, jp * 512:(jp + 1) * 512],
                            start=(k == 0), stop=(k == 63))
                    if jg % 4 == 3:
                        h = jg // 4
                        u_sb = pc.tile([64, 512], FP, tag="u_sb")
                        nc.vector.tensor_copy(u_sb[:], ups[0:64, :])
                        ta = TBL.ap()
                        nc.sync.dma_start(
                            bass.AP(ta.tensor, (iv * ROWS + h * 128) * S,
                                    [[2 * S, 64], [1, 2 * S]]),
                            u_sb[:])

            # ================= PHASE D: decode steps 1..255 =================
            if "D" in PH:
             with nc.named_scope("phaseD"), \
                 tc.tile_pool(name="pd", bufs=3) as pd, \
                 tc.tile_pool(name="pg", bufs=4) as pg:
                ch_prev = ch_i
                for t in range(1, S):
                    gt = pg.tile([BC, S], FP, tag="g")
                    nc.sync.dma_start(gt[:], G_t.ap()[t])
                    gidx = pd.tile([BC, 1], I32, tag="gidx")
                    nc.vector.tensor_tensor(out=gidx[:], in0=iota257[:], in1=ch_prev[:],
                                            op=AL.add)
                    row = pd.tile([BC, S], FP, tag="row")
                    nc.gpsimd.indirect_dma_start(
                        out=row[:], out_offset=None, in_=TBL.ap(),
                        in_offset=bass.IndirectOffsetOnAxis(ap=gidx[:, :1], axis=0))
                    mg = pd.tile([BC, S], FP, tag="mg")
                    nc.gpsimd.tensor_tensor(out=mg[:], in0=maskadd[:], in1=gt[:],
                                            op=AL.add)
                    rt = pd.tile([BC, S], FP, tag="rt")
                    nc.scalar.activation(rt[:], row[:], AF.Tanh)
                    masked = pd.tile([BC, S], FP, tag="masked")
                    nc.vector.scalar_tensor_tensor(out=masked[:], in0=rt[:], scalar=CLIP,
                                                   in1=maskadd[:], op0=AL.mult, op1=AL.add)
                    sc = pd.tile([BC, S], FP, tag="sc")
                    nc.vector.scalar_tensor_tensor(out=sc[:], in0=rt[:], scalar=CLIP,
                                                   in1=mg[:], op0=AL.mult, op1=AL.add)
                    mv = pd.tile([BC, 8], FP, tag="mv")
                    mi = pd.tile([BC, 8], U32, tag="mi")
                    nc.vector.max_with_indices(mv[:], mi[:], sc[:])
                    ch = pd.tile([BC, 1], I32, tag="ch")
                    nc.vector.tensor_copy(ch[:], mi[:, :1])
                    nc.vector.tensor_copy(idx_acc[:, t:t + 1], mi[:, :1])
                    chf = pd.tile([BC, 1], FP, tag="chf")
                    nc.gpsimd.tensor_copy(chf[:], mi[:, :1])
                    eqf = pd.tile([BC, S], FP, tag="eqf")
                    nc.vector.tensor_scalar(out=eqf[:], in0=iota_f[:], scalar1=chf[:, :1],
                                            scalar2=None, op0=AL.is_equal)
                    nc.vector.tensor_reduce(MXN[:, t:t + 1], masked[:],
                                            axis=mybir.AxisListType.X, op=AL.max,
                                            negate=True)
                    ej = pd.tile([BC, S], FP, tag="ej")
                    nc.scalar.activation(ej[:], masked[:], AF.Exp, bias=MXN[:, t:t + 1],
                                         accum_out=SE[:, t:t + 1])
                    jk = pd.tile([BC, S], FP, tag="jk")
                    nc.vector.tensor_mul(jk[:], masked[:], eqf[:])
                    nc.vector.tensor_reduce(CL[:, t:t + 1], jk[:],
                                            axis=mybir.AxisListType.X, op=AL.add)
                    nc.vector.scalar_tensor_tensor(out=maskadd[:], in0=eqf[:], scalar=NEG,
                                                    in1=maskadd[:], op0=AL.mult, op1=AL.add)
                    ch_prev = ch

            # ================= PHASE E: finalize logprobs =================
            if "E" in PH:
             with nc.named_scope("phaseE"), tc.tile_pool(name="pe", bufs=1) as pe:
                lnse = pe.tile([BC, S], FP)
                nc.scalar.activation(lnse[:], SE[:], AF.Ln)
                lp1 = pe.tile([BC, S], FP)
                nc.vector.tensor_add(lp1[:], CL[:], MXN[:])
                lp2 = pe.tile([BC, S], FP)
                nc.vector.tensor_sub(lp2[:], lp1[:], lnse[:])
                nc.sync.dma_start(lp_t.ap(), lp2[:])
                nc.sync.dma_start(idx_t.ap(), idx_acc[:])

    nc.compile()
    return nc


def _get_nc():
    if "nc" not in _CACHE:
        import concourse.bass as bass
        import concourse.mybir as mybir
        import concourse.tile as tile
        _CACHE["nc"] = _build((bass, mybir, tile))
    return _CACHE["nc"]


def kernel(**inputs):
    from concourse import bass_utils

    G, w = _host_prep(inputs)
    nc = _get_nc()

    x = np.asarray(inputs["inputs"], np.float32)          # [B, S, 2]
    in_maps = []
    for c in range(NCORES):
        sl = slice(c * BC, (c + 1) * BC)
        m = {k: w[k] for k in ("wq", "wk", "wv", "wo", "w1", "prw", "hctx", "pqw",
                               "m1", "mr", "embw", "vecs", "fb1")}
        m["w2"] = w["w2"]
        m["xT"] = np.ascontiguousarray(x[sl].transpose(0, 2, 1))
        m["G"] = np.ascontiguousarray(G[:, sl, :])
        in_maps.append(m)

    res = bass_utils.run_bass_kernel_spmd(nc, in_maps, core_ids=list(range(NCORES)))
    lp = np.concatenate([r["lp"] for r in res.results], axis=0)
    idx = np.concatenate([r["idx"] for r in res.results], axis=0)
    return lp, idx.astype(np.int32)


if __name__ == "__main__":
    import reference as ref
    import jax
    with jax.default_device(jax.devices("cpu")[0]):
        inputs = {k: np.asarray(v) for k, v in ref.setup_inputs().items()}
        exp_lp, exp_idx = ref.reference(**inputs)
        exp_lp, exp_idx = np.asarray(exp_lp), np.asarray(exp_idx)
    lp, idx = kernel(**inputs)
    n_bad = int((idx != exp_idx).any(axis=1).sum())
    print(f"rows diverging: {n_bad}/1024")
    rel_lp = np.linalg.norm(lp - exp_lp) / np.linalg.norm(exp_lp)
    rel_ix = np.linalg.norm((idx - exp_idx).astype(np.float64)) / np.linalg.norm(exp_idx.astype(np.float64))
    print(f"rel err: logprobs {rel_lp:.5f} indices {rel_ix:.5f}")
    ok = idx == exp_idx
    print(f"logprob err on matching entries: {np.abs(lp - exp_lp)[ok].max():.3e}")
